# revision 19
# baseline (speedup 1.0000x reference)
"""Performer (FAVOR+) linear attention on 8 TRN2 NeuronCores.

Sharding: core c handles batch b=c//4 and head group g=c%4 (4 of 16 heads).
Host converts inputs to bf16, sums the 4 per-batch partials and adds bias.

v2 design (vs fp32r baseline at 1.385 ms):
 - all PE operands bf16 (1 cycle/col at any free dim, FWL weight loads);
   end-to-end error vs f32 reference ~7e-3 (measured in numpy).
 - qkv stays resident in SBUF (no DRAM round-trip).
 - context accumulated transposed (ctxT = vx.T @ kp): stationary vx,
   moving kp [n,267] -> one 111ns matmul per tile instead of 3 LDW-bound
   chunk matmuls.
 - output accumulated transposed (oeT = ctx.T @ qp.T): stationary ctx
   chunks, moving qpT 512 wide.
 - eps floor handled exactly via rank-1 corrections (matmul with ones /
   K=1 outer products); q-side rowmax and k-side global max over the raw
   dash match the reference eps semantics exactly.
 - per-head D row scaled via broadcast matmul (E2 @ D) + one DVE mult.
"""
import sys
sys.path.insert(0, '/opt/trn_rl_repo')

import numpy as np
import concourse.bass as bass
import concourse.bacc as bacc
import concourse.tile as tile
from concourse import mybir
from concourse.bass_utils import run_bass_kernel_spmd

F32 = mybir.dt.float32
F32R = mybir.dt.float32r
BF16 = mybir.dt.bfloat16
AX = mybir.AxisListType.X
AF = mybir.ActivationFunctionType
ALU = mybir.AluOpType

B, N, D = 2, 4096, 1024
H, DH, M = 16, 64, 266          # heads, dim_head, nb_features
HPC = 4                         # heads per core
EPS = 1e-4
CNORM = DH ** -0.25
RATIO = M ** -0.5
LNR = float(np.log(RATIO))
RE = float(RATIO * EPS)
NT = N // 128                   # 32 n-tiles
NB = N // 512                   # 8 n-blocks
MCH = [(0, 128), (128, 128), (256, 10)]   # m-chunks of 266
CN2 = float(np.sqrt(0.5) * CNORM)         # scale so Square-accum yields 0.5*c^2*sum(k^2)


def build():
    nc = bacc.Bacc("TRN2", target_bir_lowering=False, debug=False)

    xT = nc.dram_tensor("xT", [D, N], BF16, kind="ExternalInput")
    wqkT = nc.dram_tensor("wqkT", [D, 512], BF16, kind="ExternalInput")
    wvT = nc.dram_tensor("wvT", [D, 256], BF16, kind="ExternalInput")
    woP = nc.dram_tensor("woP", [128, 2048], BF16, kind="ExternalInput")
    projc = nc.dram_tensor("projc", [DH, M], BF16, kind="ExternalInput")
    ident = nc.dram_tensor("ident", [128, 128], BF16, kind="ExternalInput")
    identf = nc.dram_tensor("identf", [128, 128], F32, kind="ExternalInput")
    e2d = nc.dram_tensor("e2d", [2, 128], F32, kind="ExternalInput")
    y = nc.dram_tensor("y", [N, D], BF16, kind="ExternalOutput")

    with tile.TileContext(nc) as tc:
        with tc.tile_pool(name="const", bufs=1) as cpool, \
             tc.tile_pool(name="big", bufs=1) as big, \
             tc.tile_pool(name="strm", bufs=3) as strm, \
             tc.tile_pool(name="sml", bufs=4) as sml, \
             tc.tile_pool(name="psA", bufs=2, space="PSUM") as psA, \
             tc.tile_pool(name="psV", bufs=2, space="PSUM") as psV, \
             tc.tile_pool(name="psD", bufs=2, space="PSUM") as psD, \
             tc.tile_pool(name="psC", bufs=2, space="PSUM") as psC:

            # ---- constants / weights ----
            wqk = cpool.tile([128, 8, 512], BF16, tag="wqk")
            nc.sync.dma_start(wqk[:], wqkT.ap().rearrange("(c p) n -> p c n", p=128))
            wv = cpool.tile([128, 8, 256], BF16, tag="wv")
            nc.sync.dma_start(wv[:], wvT.ap().rearrange("(c p) n -> p c n", p=128))
            wo = cpool.tile([128, 2048], BF16, tag="wo")
            nc.sync.dma_start(wo[:], woP.ap())
            pj = cpool.tile([128, M], BF16, tag="pj")
            nc.sync.dma_start(pj[0:64, :], projc.ap())
            nc.sync.dma_start(pj[64:128, :], projc.ap())
            identb = cpool.tile([128, 128], BF16, tag="idb")
            nc.sync.dma_start(identb[:], ident.ap())
            identft = cpool.tile([128, 128], F32, tag="idf")
            nc.sync.dma_start(identft[:], identf.ap())
            ones1f = cpool.tile([1, 128], F32, tag="o1f")
            nc.vector.memset(ones1f[:], 1.0)
            ones1b = cpool.tile([1, 128], BF16, tag="o1b")
            nc.vector.memset(ones1b[:], 1.0)
            onesr512 = cpool.tile([1, 512], BF16, tag="o512")
            nc.vector.memset(onesr512[:], 1.0)
            onescol = cpool.tile([128, 1], BF16, tag="ocol")
            nc.vector.memset(onescol[:], 1.0)
            e2a = cpool.tile([1, 128], F32, tag="e2a")
            nc.sync.dma_start(e2a[:], e2d.ap()[0:1, :])
            e2b = cpool.tile([1, 128], F32, tag="e2b")
            nc.sync.dma_start(e2b[:], e2d.ap()[1:2, :])
            e2ar = cpool.tile([1, 128], F32R, tag="e2ar")
            nc.scalar.copy(e2ar[:], e2a[:])
            e2br = cpool.tile([1, 128], F32R, tag="e2br")
            nc.scalar.copy(e2br[:], e2b[:])

            qall = big.tile([128, NT, 256], BF16, tag="qall")
            kall = big.tile([128, NT, 256], BF16, tag="kall")
            qTall = big.tile([128, NT, 2, 128], BF16, tag="qTall")
            kTall = big.tile([128, NT, 2, 128], BF16, tag="kTall")
            vxall = big.tile([128, NT, HPC, 66], BF16, tag="vx")
            nc.vector.memset(vxall[:, :, :, 64:66], 1.0)
            dashkb = big.tile([128, NT, M], F32, tag="dashk")
            otbs = big.tile([128, 2, N], BF16, tag="otb")
            dpair = big.tile([1, N], F32R, tag="dpair")

            # ---- phase 1: QKV projections into SBUF ----
            for j in range(NT):
                xt = strm.tile([128, 8, 128], BF16, tag="xt")
                nc.sync.dma_start(
                    xt[:], xT.ap().rearrange("(c p) n -> p c n", p=128)[:, :, j*128:(j+1)*128])
                qk_ps = psA.tile([128, 512], F32, tag="dash", name=f"qk{j}")
                for c in range(8):
                    nc.tensor.matmul(qk_ps[:], xt[:, c, :], wqk[:, c, :],
                                     start=(c == 0), stop=(c == 7))
                v_ps = psV.tile([128, 256], F32, tag="pv", name=f"v{j}")
                for c in range(8):
                    nc.tensor.matmul(v_ps[:], xt[:, c, :], wv[:, c, :],
                                     start=(c == 0), stop=(c == 7))
                nc.any.tensor_copy(qall[:, j, :], qk_ps[:, 0:256])
                nc.any.tensor_copy(kall[:, j, :], qk_ps[:, 256:512])
                nc.any.tensor_copy(vxall[:, j, :, 0:64],
                                   v_ps[:].rearrange("p (h e) -> p h e", e=64))
                for g in range(2):
                    qt_ps = psV.tile([128, 128], BF16, tag="pv", name=f"qt{j}_{g}")
                    nc.tensor.transpose(qt_ps[:], qall[:, j, g*128:(g+1)*128],
                                        identb[:])
                    nc.any.tensor_copy(qTall[:, j, g, :], qt_ps[:])
                    kt_ps = psV.tile([128, 128], BF16, tag="pv", name=f"kt{j}_{g}")
                    nc.tensor.transpose(kt_ps[:], kall[:, j, g*128:(g+1)*128],
                                        identb[:])
                    nc.any.tensor_copy(kTall[:, j, g, :], kt_ps[:])

            # ---- per-head chain ----
            for h in range(HPC):
                hs = h * 64
                # K1: dash_k tiles + row max + diag
                rmaxb = sml.tile([128, NT], F32, tag="rmaxb", bufs=2)
                diagkb = sml.tile([128, NT], F32, tag="diagkb", bufs=2)
                pb, ch = (h % 2) * 64, h // 2
                for j in range(NT):
                    sqj = strm.tile([128, DH], BF16, tag="sqj", bufs=2)
                    nc.gpsimd.tensor_mul(sqj[:], kall[:, j, hs:hs+64],
                                         kall[:, j, hs:hs+64])
                    nc.vector.tensor_reduce(diagkb[:, j:j+1], sqj[:], axis=AX,
                                            op=ALU.add)
                    dash_ps = psA.tile([128, M], F32, tag="dash", name=f"dk{h}_{j}")
                    nc.tensor.matmul(dash_ps[:], kTall[pb:pb+64, j, ch, :],
                                     pj[pb:pb+64, :], start=True, stop=True)
                    nc.vector.tensor_copy(dashkb[:, j, :], dash_ps[:])
                    nc.vector.reduce_max(rmaxb[:, j:j+1], dashkb[:, j, :], axis=AX)

                # global max of raw dash_k -> per-partition bias column
                gmax = sml.tile([128, 1], F32, tag="gmax")
                nc.vector.reduce_max(gmax[:], rmaxb[:], axis=AX)
                gm_ps = psV.tile([1, 128], F32, tag="pv", name=f"gm{h}")
                nc.tensor.transpose(gm_ps[:], gmax[:], identft[:])
                gmrow = sml.tile([1, 128], F32, tag="gmrow")
                nc.any.tensor_copy(gmrow[:], gm_ps[:])
                mk = sml.tile([1, 1], F32, tag="mk")
                nc.vector.reduce_max(mk[:], gmrow[:], axis=AX)
                mk_ps = psV.tile([128, 1], F32, tag="pv", name=f"mkb{h}")
                nc.tensor.matmul(mk_ps[:], ones1f[:], mk[:], start=True, stop=True)
                mkl = sml.tile([128, 1], F32, tag="mkl")
                nc.vector.tensor_scalar(mkl[:], mk_ps[:], -1.0, LNR,
                                        op0=ALU.mult, op1=ALU.add)
                biaskb = sml.tile([128, NT], F32, tag="biaskb", bufs=2)
                nc.vector.tensor_scalar(biaskb[:], diagkb[:],
                                        -0.5 * CNORM * CNORM, mkl[:],
                                        op0=ALU.mult, op1=ALU.add)

                # K2: kp = exp(dash - diag - mk + lnr); ctxT accumulation
                ctxT_ps = psD.tile([66, M + 1], F32, tag="ctxT", name=f"ctxT{h}")
                for j in range(NT):
                    kp = strm.tile([128, M + 1], BF16, tag="kp")
                    nc.scalar.activation(kp[:, 1:M+1], dashkb[:, j, :], AF.Exp,
                                         bias=biaskb[:, j:j+1], scale=1.0)
                    nc.vector.memset(kp[:, 0:1], 1.0)
                    nc.tensor.matmul(ctxT_ps[:], vxall[:, j, h, :], kp[:],
                                     start=(j == 0), stop=(j == NT - 1))

                # ctx chunks [m,66] with eps correction (+ RE * colsum_vx per col)
                ctxs = strm.tile([66, M + 1], BF16, tag="ctxs", bufs=2)
                nc.any.tensor_copy(ctxs[:], ctxT_ps[:])
                cv_ps = psV.tile([1, 66], BF16, tag="pv", name=f"cv{h}")
                nc.tensor.transpose(cv_ps[:], ctxs[:, 0:1], identb[0:66, 0:66])
                epsrow = sml.tile([1, 66], BF16, tag="epsrow", bufs=2)
                nc.scalar.mul(epsrow[:], cv_ps[:], RE)
                ep_ps = psV.tile([128, 66], F32, tag="pv", name=f"ep{h}")
                nc.tensor.matmul(ep_ps[:], ones1b[:], epsrow[:], start=True, stop=True)
                epsbc = sml.tile([128, 66], F32, tag="epsbc", bufs=2)
                nc.any.tensor_copy(epsbc[:], ep_ps[:])
                ctxc = strm.tile([128, 3, 66], BF16, tag="ctxc", bufs=2)
                for mc, (off, w) in enumerate(MCH):
                    cc_ps = psV.tile([128, 128], BF16, tag="pv", name=f"cc{h}_{mc}")
                    nc.tensor.transpose(cc_ps[0:w, 0:66], ctxs[:, 1+off:1+off+w],
                                        identb[0:66, 0:66])
                    nc.vector.tensor_add(ctxc[0:w, mc, :], cc_ps[0:w, 0:66],
                                         epsbc[0:w, :])
                # colsum of corrected ctx (for the q-side eps term)
                csc_ps = psV.tile([1, 66], F32, tag="pv", name=f"csc{h}")
                for mc, (off, w) in enumerate(MCH):
                    nc.tensor.matmul(csc_ps[:], onescol[0:w, :], ctxc[0:w, mc, :],
                                     start=(mc == 0), stop=(mc == 2))
                cscrow = sml.tile([1, 66], BF16, tag="cscrow", bufs=2)
                nc.scalar.mul(cscrow[:], csc_ps[:], RE)

                # Q pass: dash, rowmax, exp, transpose, oeT blocks
                qptb = None
                for j in range(NT):
                    sqq = strm.tile([128, DH], BF16, tag="sqj", bufs=2)
                    diagq = sml.tile([128, 1], F32, tag="diagq")
                    nc.gpsimd.tensor_mul(sqq[:], qall[:, j, hs:hs+64],
                                         qall[:, j, hs:hs+64])
                    nc.vector.tensor_reduce(diagq[:], sqq[:], axis=AX, op=ALU.add)
                    dq_ps = psA.tile([128, M], F32, tag="dash", name=f"dq{h}_{j}")
                    nc.tensor.matmul(dq_ps[:], qTall[pb:pb+64, j, ch, :],
                                     pj[pb:pb+64, :], start=True, stop=True)
                    rmaxq = sml.tile([128, 1], F32, tag="rmaxq")
                    nc.vector.reduce_max(rmaxq[:], dq_ps[:], axis=AX)
                    biasq = sml.tile([128, 1], F32, tag="biasq")
                    nc.vector.tensor_scalar(biasq[:], diagq[:],
                                            -0.5 * CNORM * CNORM, LNR,
                                            op0=ALU.mult, op1=ALU.add)
                    nc.vector.tensor_sub(biasq[:], biasq[:], rmaxq[:])
                    qp = strm.tile([128, 384], BF16, tag="qp")
                    nc.scalar.activation(qp[:, 0:M], dq_ps[:], AF.Exp,
                                         bias=biasq[:], scale=1.0)
                    jj, bb = j % 4, j // 4
                    if jj == 0:
                        qptb = strm.tile([128, 3, 512], BF16, tag="qptb", bufs=2)
                    qpt_ps = psV.tile([128, 384], BF16, tag="pv", name=f"qp{h}_{j}")
                    for mc, (off, w) in enumerate(MCH):
                        nc.tensor.transpose(qpt_ps[0:w, mc*128:(mc+1)*128],
                                            qp[:, off:off+w], identb[:])
                    nc.any.tensor_copy(
                        qptb[:, 0:2, jj*128:(jj+1)*128],
                        qpt_ps[:, 0:256].rearrange("p (c n) -> p c n", n=128))
                    nc.any.tensor_copy(qptb[0:10, 2, jj*128:(jj+1)*128],
                                       qpt_ps[0:10, 256:384])
                    if jj == 3:
                        oeT_ps = psC.tile([66, 512], F32, tag="oeT", name=f"oe{h}_{bb}")
                        for mc, (off, w) in enumerate(MCH):
                            nc.tensor.matmul(oeT_ps[:], ctxc[0:w, mc, :],
                                             qptb[0:w, mc, :],
                                             start=(mc == 0), stop=False)
                        nc.tensor.matmul(oeT_ps[:], cscrow[:], onesr512[:],
                                         start=False, stop=True)
                        pb, ch = (h % 2) * 64, h // 2
                        sl = slice(bb*512, (bb+1)*512)
                        nc.any.tensor_copy(
                            otbs[pb:pb+64, ch, sl], oeT_ps[0:64, :])
                        if h % 2 == 0:
                            nc.any.tensor_copy(dpair[:, sl], oeT_ps[64:65, :])
                        else:
                            # normalize both heads of the pair for this block
                            dr = sml.tile([1, 512], F32R, tag="dr", bufs=2)
                            nc.any.tensor_copy(dr[:], oeT_ps[64:65, :])
                            db_ps = psV.tile([128, 512], F32, tag="pv",
                                             name=f"db{h}_{bb}")
                            nc.tensor.matmul(db_ps[:], e2ar[:], dpair[:, sl],
                                             start=True, stop=False)
                            nc.tensor.matmul(db_ps[:], e2br[:], dr[:],
                                             start=False, stop=True)
                            dinvb = sml.tile([128, 512], F32, tag="dinvb", bufs=2)
                            nc.vector.reciprocal_approx_fast(dinvb[:], db_ps[:])
                            nc.vector.tensor_mul(otbs[:, ch, sl],
                                                 otbs[:, ch, sl], dinvb[:])

            # ---- phase 3: output projection ----
            for j in range(NT):
                y_ps = [psA.tile([128, 512], F32, tag="dash", name=f"y{j}_{nb}")
                        for nb in range(2)]  # noqa: name passed explicitly
                for nb in range(2):
                    for ch in range(2):
                        nc.tensor.matmul(y_ps[nb][:],
                                         otbs[:, ch, j*128:(j+1)*128],
                                         wo[:, ch*1024 + nb*512:
                                            ch*1024 + nb*512 + 512],
                                         start=(ch == 0), stop=(ch == 1))
                y_s = strm.tile([128, 1024], BF16, tag="ys", bufs=2)
                for nb in range(2):
                    nc.any.tensor_copy(y_s[:, nb*512:(nb+1)*512], y_ps[nb][:])
                nc.sync.dma_start(y.ap()[j*128:(j+1)*128, :], y_s[:])

    nc.compile()
    return nc


_prog = None


def _bf16(a):
    from ml_dtypes import bfloat16
    return np.ascontiguousarray(np.asarray(a, np.float32)).astype(bfloat16)


def _build_in_maps(inputs):
    return _make_in_maps(**inputs)


def _make_in_maps(x, Wq, Wk, Wv, Wo, bo, proj):
    x = np.asarray(x, np.float32)
    Wq = np.asarray(Wq, np.float32)
    Wk = np.asarray(Wk, np.float32)
    Wv = np.asarray(Wv, np.float32)
    Wo = np.asarray(Wo, np.float32)
    projc = np.ascontiguousarray(CNORM * np.asarray(proj, np.float32).T)
    identm = np.eye(128, dtype=np.float32)
    e2m = np.zeros((2, 128), np.float32)
    e2m[0, 0:64] = 1.0
    e2m[1, 64:128] = 1.0
    xTb = [np.ascontiguousarray(x[b].T) for b in range(B)]
    in_maps = []
    for c in range(8):
        b, g = c // 4, c % 4
        hs, he = g * 256, g * 256 + 256
        woT = Wo[:, hs:he].T                                   # [256, 1024]
        woP = np.concatenate([woT[:128], woT[128:]], axis=1)   # [128, 2048]
        wqkT = np.concatenate([Wq[hs:he].T, Wk[hs:he].T], axis=1)  # [1024, 512]
        in_maps.append({
            "xT": _bf16(xTb[b]),
            "wqkT": _bf16(wqkT),
            "wvT": _bf16(Wv[hs:he].T),
            "woP": _bf16(woP),
            "projc": _bf16(projc),
            "ident": _bf16(identm),
            "identf": identm,
            "e2d": e2m,
        })
    return in_maps


def kernel(x, Wq, Wk, Wv, Wo, bo, proj):
    global _prog
    if _prog is None:
        _prog = build()
    in_maps = _make_in_maps(x, Wq, Wk, Wv, Wo, bo, proj)
    res = run_bass_kernel_spmd(_prog, in_maps, core_ids=list(range(8)))
    out = np.zeros((B, N, D), np.float32)
    for c in range(8):
        out[c // 4] += np.asarray(res.results[c]["y"], np.float32)
    out += np.asarray(bo, np.float32)[None, None, :]
    return out
